# revision 10
# baseline (speedup 1.0000x reference)
"""BatchHardTripletLoss on 8 TRN2 NeuronCores (Bass/Tile).

Contract: kernel(**inputs) takes the FULL inputs (h1,h2,h3: [2048,512] f32)
and returns the full output tuple (loss, mean_diff, good, bad, rms_norm)
matching reference semantics:

    batch = concat(h1, h2)            # [4096, 512]
    d2[i,j] = sq[i] + sq[j] - 2 * (batch @ batch.T)[i,j]
    d = sqrt(max(d2, 1e-14)); d = max(d, 1e-7)
    hp[i] = d[i, (i+2048) % 4096]                  # the single positive
    hn[i] = min_{j not in {i, partner}} d[i, j]    # hardest negative

Sharding: rows (anchors) split 512/core across 8 cores. Each core gets a
column-ROTATED copy of batch.T (rolled by -512*core) so the kernel is
fully SPMD-static: its own diagonal block is always column-tile 0 and the
positive-pair block is always column-tile 4, with the excluded column at
static in-tile offset 128*m + p for row-chunk m, partition p.

Mining happens on f[i,j] = g[i,j] - sq[j]/2 straight out of PSUM
(argmin of d2 = argmax of f), so the heavy [128,512] traffic is one fused
tensor_tensor_reduce per tile; sqrt/clamps only touch [128,1] vectors.
"""

import os
import sys

import numpy as np

if "/opt/trn_rl_repo" not in sys.path:
    sys.path.insert(0, "/opt/trn_rl_repo")

N = 2048
TN = 2 * N          # 4096 rows in the distance matrix
D = 512             # feature dim
NCORES = 8
RB = TN // NCORES   # 512 rows per core
MCH = RB // 128     # 4 row-chunks of 128 per core
NT = TN // 512      # 8 column tiles of 512
KT = D // 128       # 4 contraction tiles of 128
NEG_BIG = -1.0e30

# Matmul input dtype: "f32r" (full PE rate, ~fp32 inputs), "f32" (4x slower,
# exact fp32), or "bf16" (full rate, reduced precision + half DMA if used
# with bf16 staging — not wired up).
MM_DTYPE = os.environ.get("BASS_MM_DTYPE", "f32r")

_CACHE = {}

# test.py introspection: exec time of the last hardware run (ns) when
# BASS_KERNEL_TRACE=1, else None.
last_exec_ns = None
last_profile_json = None


def _build_nc():
    import concourse.bacc as bacc
    import concourse.mybir as mybir
    from concourse.tile import TileContext

    f32 = mybir.dt.float32
    mm_dt = {
        "f32r": mybir.dt.float32r,
        "f32": mybir.dt.float32,
        "bf16": mybir.dt.bfloat16,
    }[MM_DTYPE]
    Alu = mybir.AluOpType
    Ax = mybir.AxisListType

    nc = bacc.Bacc("TRN2", target_bir_lowering=False, debug=False)

    bt = nc.declare_dram_parameter("bt", [D, TN], mm_dt, isOutput=False)
    sqh = nc.declare_dram_parameter("sqh", [1, TN], f32, isOutput=False)
    sqr = nc.declare_dram_parameter("sqr", [RB, 1], f32, isOutput=False)
    out = nc.declare_dram_parameter("out", [RB, 2], f32, isOutput=True)

    with TileContext(nc) as tc:
        with (
            tc.tile_pool(name="persist", bufs=1) as pp,
            tc.tile_pool(name="psum", bufs=6, space="PSUM") as psp,
            tc.tile_pool(name="work", bufs=4) as wp,
            tc.tile_pool(name="small", bufs=8) as sp,
        ):
            # --- loads -------------------------------------------------
            btk = []
            for k in range(KT):
                t = pp.tile([128, TN], mm_dt, name=f"btk{k}")
                btk.append(t)
            # Column-chunked so compute on early column tiles can start
            # while later chunks are still in flight.
            for c in range(NT):
                for k in range(KT):
                    nc.sync.dma_start(
                        out=btk[k][:, 512 * c : 512 * (c + 1)],
                        in_=bt[128 * k : 128 * (k + 1), 512 * c : 512 * (c + 1)],
                    )

            sqh_row = pp.tile([1, TN], f32, name="sqh_row")
            nc.sync.dma_start(out=sqh_row[:, :], in_=sqh[0:1, :])
            sqhb = pp.tile([128, TN], f32, name="sqhb")
            if os.environ.get("BASS_NO_PB", "0") == "1":
                nc.sync.dma_start(
                    out=sqhb[:, :], in_=sqh[0:1, :].partition_broadcast(128)
                )
            else:
                nc.gpsimd.partition_broadcast(sqhb[:, :], sqh_row[:, :])

            sqrows = pp.tile([128, MCH], f32, name="sqrows")
            for m in range(MCH):
                nc.sync.dma_start(
                    out=sqrows[:, m : m + 1],
                    in_=sqr[128 * m : 128 * (m + 1), 0:1],
                )

            # --- main grid ---------------------------------------------
            for m in range(MCH):
                pmax = sp.tile([128, NT], f32, name="pmax", tag="pmax")
                fpart = sp.tile([128, 1], f32, name="fpart", tag="fpart")
                for n in range(NT):
                    ps = psp.tile([128, 512], f32, name="ps", tag="ps")
                    for k in range(KT):
                        nc.tensor.matmul(
                            ps[:, :],
                            btk[k][:, 128 * m : 128 * (m + 1)],
                            btk[k][:, 512 * n : 512 * (n + 1)],
                            start=(k == 0),
                            stop=(k == KT - 1),
                        )
                    sqh_sl = sqhb[:, 512 * n : 512 * (n + 1)]
                    if n != 0 and n != NT // 2:
                        # plain tile: f = g - sqh ; pmax[:,n] = max(f) fused
                        f = wp.tile([128, 512], f32, name="f", tag="f")
                        if os.environ.get("BASS_USE_TTR", "0") != "1":
                            nc.vector.tensor_tensor(
                                f[:, :], ps[:, :], sqh_sl, op=Alu.subtract
                            )
                            nc.vector.tensor_reduce(
                                out=pmax[:, n : n + 1],
                                in_=f[:, :],
                                axis=Ax.X,
                                op=Alu.max,
                            )
                        else:
                            nc.vector.tensor_tensor_reduce(
                                out=f[:, :],
                                in0=ps[:, :],
                                in1=sqh_sl,
                                scale=1.0,
                                scalar=NEG_BIG,
                                op0=Alu.subtract,
                                op1=Alu.max,
                                accum_out=pmax[:, n : n + 1],
                            )
                    else:
                        # special tile: excluded column at 128*m + p
                        f = wp.tile([128, 512], f32, name="f", tag="f")
                        nc.vector.tensor_tensor(
                            f[:, :], ps[:, :], sqh_sl, op=Alu.subtract
                        )
                        fx = wp.tile([128, 512], f32, name="fx", tag="fx")
                        nc.gpsimd.affine_select(
                            out=fx[:, :],
                            in_=f[:, :],
                            pattern=[[1, 512]],
                            compare_op=Alu.not_equal,
                            fill=NEG_BIG,
                            base=-128 * m,
                            channel_multiplier=-1,
                        )
                        nc.vector.tensor_reduce(
                            out=pmax[:, n : n + 1],
                            in_=fx[:, :],
                            axis=Ax.X,
                            op=Alu.max,
                        )
                        if n == NT // 2:
                            # positive pair value f[i, partner]
                            fpx = wp.tile([128, 512], f32, name="fpx", tag="fx")
                            nc.gpsimd.affine_select(
                                out=fpx[:, :],
                                in_=f[:, :],
                                pattern=[[1, 512]],
                                compare_op=Alu.is_equal,
                                fill=NEG_BIG,
                                base=-128 * m,
                                channel_multiplier=-1,
                            )
                            nc.vector.tensor_reduce(
                                out=fpart[:, :],
                                in_=fpx[:, :],
                                axis=Ax.X,
                                op=Alu.max,
                            )

                fmax = sp.tile([128, 1], f32, name="fmax", tag="fmax")
                nc.vector.tensor_reduce(
                    out=fmax[:, :], in_=pmax[:, :], axis=Ax.X, op=Alu.max
                )
                # d2 = sq_i - 2 * f ; d = sqrt(max(d2, 1e-14))
                hpn = sp.tile([128, 2], f32, name="hpn", tag="hpn")
                sq_m = sqrows[:, m : m + 1]
                for col, src in ((0, fpart), (1, fmax)):
                    d2 = sp.tile([128, 1], f32, name=f"d2_{col}", tag=f"d2{col}")
                    nc.vector.scalar_tensor_tensor(
                        out=d2[:, :],
                        in0=src[:, :],
                        scalar=-2.0,
                        in1=sq_m,
                        op0=Alu.mult,
                        op1=Alu.add,
                    )
                    nc.vector.tensor_scalar_max(d2[:, :], d2[:, :], 1e-14)
                    nc.scalar.sqrt(hpn[:, col : col + 1], d2[:, :])

                nc.sync.dma_start(
                    out=out[128 * m : 128 * (m + 1), :], in_=hpn[:, :]
                )

    nc.finalize()
    return nc


def _get_nc():
    if "nc" not in _CACHE:
        _CACHE["nc"] = _build_nc()
    return _CACHE["nc"]


def kernel(h1, h2, h3=None, **_unused):
    global last_exec_ns, last_profile_json
    from concourse.bass_utils import run_bass_kernel_spmd

    h1 = np.asarray(h1, dtype=np.float32)
    h2 = np.asarray(h2, dtype=np.float32)
    batch = np.concatenate([h1, h2], axis=0)               # [4096, 512]
    bt = np.ascontiguousarray(batch.T)                     # [512, 4096]
    sq = np.sum(batch * batch, axis=1, dtype=np.float32)   # [4096]

    in_maps = []
    for c in range(NCORES):
        r0 = RB * c
        in_maps.append(
            {
                "bt": np.roll(bt, -r0, axis=1),
                "sqh": (np.roll(sq, -r0) * np.float32(0.5))[None, :],
                "sqr": np.ascontiguousarray(sq[r0 : r0 + RB][:, None]),
            }
        )

    nc = _get_nc()
    trace = os.environ.get("BASS_KERNEL_TRACE", "0") == "1"
    res = run_bass_kernel_spmd(nc, in_maps, list(range(NCORES)), trace=trace)
    last_exec_ns = res.exec_time_ns
    last_profile_json = res.profile_json

    outs = [res.results[c]["out"] for c in range(NCORES)]
    hp = np.concatenate([o[:, 0] for o in outs])           # [4096]
    hn = np.concatenate([o[:, 1] for o in outs])

    diff = (hp - hn).astype(np.float32)
    tl = np.maximum(diff + np.float32(0.1), np.float32(0.0))
    rel = tl > np.float32(1e-5)
    good = np.int32(np.sum(tl < np.float32(1e-5)))
    bad = np.int32(TN - good)
    n_rel = max(int(np.sum(rel)), 1)
    mean_rel = np.float32(np.sum(np.where(rel, tl, np.float32(0.0))) / n_rel)
    mean_diff = np.float32(np.mean(diff))
    rms = np.float32(np.sqrt(np.mean(sq)))
    loss = mean_rel
    return (loss, mean_diff, good, bad, rms)
